# revision 53
# baseline (speedup 1.0000x reference)
"""Trainium2 Bass kernel for nn_AttentionBlock (B=8, L=2048, E=512, FF=2048).

Strategy: data-parallel over batch — core b computes batch item b end-to-end
(no collectives). All activations live transposed ([feature, token], feature on
partitions) so every matmul contracts over the partition dim.

The large matmuls run in fp8 with perf_mode=DoubleRow (the PE packs 2 fp8
weights per cell, contracting 256 rows per pass — ~2x the fp16/f32r rate):

  - Attention is folded on the host: M = 128*SCALE*(Wq^T Wk) so a single
    projection mq = M^T x replaces both q and k projections, and scores are
    x8^T mq directly against the fp8 copy of x. Softmax weights are stored
    e5m2 (scores reach ~12.6, so the fixed bias exp(s - 7*ln2) needs e5m2's
    range; everything else is e4m3). The v projection is interleaved into
    chunk 0's score stream so it runs behind the initial x8/wv8 DMA.
  - FFN1 runs fp8-DoubleRow (h copied to e4m3, w1 host-scaled by 32); FFN2
    runs a DR_FO-controlled mix of fp8-DoubleRow and fp16 — full fp8 there
    would push the output error to 1.7e-2 against the 2e-2 gate. The 2^-11
    descale is fused into the residual add (affine_then_add).
  - LN statistics also run fp8-DoubleRow against fp8 shadows of y/y^2 (mean
    and variance tolerate 2.4%/sqrt(512) noise); the residual stream itself
    (xt, y2) stays f32r — that is what the output precision depends on.
    Keeping f32r matmuls out of the DR stream also matters for speed: a
    f32r matmul sandwiched between DR ones measures ~2.2x its solo cost.

Scheduling notes, each worth real ns on the PE critical path:
  - DMA issue ops live on the sync and gpsimd queues only — the scalar
    (ACT) queue must stay free for PSUM evictions, which gate the PE's
    software pipeline (13 queued dma_starts once cost a 21us PE stall).
  - fp8 shadow copies run on the DVE (gpsimd's tensor_copy is ~2us/tile,
    6x the DVE) as single 3D-AP ops; squares on the ACT, except inside
    phase B where the ACT is saturated by exp evictions.
  - resid2 is split: its DVE/ACT part ends chunk i, its PE stats matmuls
    are deferred into chunk i+1's FFN stream (like ln2 of chunk i-1), so
    the PE never idles on the shadow chain at chunk seams. The last chunk
    runs resid2+ln2 in two half-width pieces to pipeline the tail.
"""
import math
from contextlib import ExitStack

import ml_dtypes
import numpy as np

import concourse.bass as bass
import concourse.bacc as bacc
import concourse.tile as tile
from concourse import mybir
from concourse.bass_utils import run_bass_kernel_spmd

P = 128
B, L, E, FF = 8, 2048, 512, 2048
NDOM = 32
EPS = 1e-5
SCALE = (1.0 / math.sqrt(E)) * 2.0 * math.log(NDOM)
EXPB = -7.0 * math.log(2.0)   # exp(s+EXPB): keeps e5m2 weights in range

EO = E // P           # 4  e-chunks
EP = EO // 2          # 2  e-chunk pairs (DoubleRow)
FO = FF // P          # 16 f-chunks
LC = 512              # l-chunk (matmul free dim)
NLC = L // LC         # 4  l-chunks
SB = L // P           # 16 s-blocks
SBP = SB // 2         # 8  s-block pairs

W1S = 32.0            # host scale on w1 (keeps fp8 weights normal-range)
W2S = 64.0            # host scale on w2 (both fp8 and f16 parts, so the
                      # mixed-precision FFN2 accumulates at one scale)
WVS = 64.0            # host scale on wv
MS = 128.0            # host scale on M = SCALE*Wq^T Wk
RESID_SCALE = 1.0 / (W1S * W2S)   # descale fused into the FFN residual add

# FFN2 precision mix: the first 2*DR_FO f-blocks multiply in fp8 DoubleRow
# (fast, adds quantization noise), the rest in fp16. Measured output rel err
# vs DR_FO: 0 -> 1.24e-2, 2 -> ~1.38e-2, 4 -> ~1.50e-2, 8 -> 1.72e-2
# against the 2e-2 gate.
DR_FO = 2

F32 = mybir.dt.float32
F32R = mybir.dt.float32r
F16 = mybir.dt.float16
F8 = mybir.dt.float8e4
F8E5 = mybir.dt.float8e5
AF = mybir.ActivationFunctionType
OP = mybir.AluOpType
DR = mybir.MatmulPerfMode.DoubleRow

_TRACE = False
LAST_RESULT = None
_CACHE = {}


def _round_fp32r(x):
    """Round-to-nearest-even fp32 -> fp32r (low 12 mantissa bits cleared)."""
    u = np.ascontiguousarray(x, dtype=np.float32).view(np.uint32)
    frac = u & np.uint32(0xFFF)
    base = u & np.uint32(0xFFFFF000)
    up = (frac > 0x800) | ((frac == 0x800) & (((u >> 12) & 1) == 1))
    return (base + np.where(up, np.uint32(0x1000), np.uint32(0))).view(np.float32)


def _to_f8(x):
    a = np.clip(np.ascontiguousarray(x, dtype=np.float32), -240.0, 240.0)
    return a.astype(ml_dtypes.float8_e4m3)


def _build(ln1_trivial, ln2_trivial, b2_zero):
    nc = bacc.Bacc("TRN2", debug=False, target_bir_lowering=False, num_devices=B)

    xt_d = nc.dram_tensor("xt", [E, L], F32R, kind="ExternalInput")
    x8_d = nc.dram_tensor("x8", [E, L], F8, kind="ExternalInput")
    m8_d = nc.dram_tensor("m8", [E, E], F8, kind="ExternalInput")
    wv8_d = nc.dram_tensor("wv8", [E, E], F8, kind="ExternalInput")
    w18_d = nc.dram_tensor("w18", [E, FF], F8, kind="ExternalInput")
    w2h_d = nc.dram_tensor("w2h", [FF, E], F16, kind="ExternalInput")
    w28_d = (nc.dram_tensor("w28", [2 * DR_FO * P, E], F8, kind="ExternalInput")
             if DR_FO else None)
    b1_d = nc.dram_tensor("b1v", [FF], F32, kind="ExternalInput")
    b2_d = None if b2_zero else nc.dram_tensor("b2v", [E], F32, kind="ExternalInput")
    ln1w_d = ln1b_d = ln2w_d = ln2b_d = None
    if not ln1_trivial:
        ln1w_d = nc.dram_tensor("ln1w", [E], F32, kind="ExternalInput")
        ln1b_d = nc.dram_tensor("ln1b", [E], F32, kind="ExternalInput")
    if not ln2_trivial:
        ln2w_d = nc.dram_tensor("ln2w", [E], F32, kind="ExternalInput")
        ln2b_d = nc.dram_tensor("ln2b", [E], F32, kind="ExternalInput")
    out_d = nc.dram_tensor("outt", [E, L], F32, kind="ExternalOutput")

    xt_r = xt_d.ap().rearrange("(eo p) l -> p eo l", p=P)
    x8_r = x8_d.ap().rearrange("(eo p) l -> p eo l", p=P)
    m8_r = m8_d.ap().rearrange("(eo p) f -> p eo f", p=P)
    wv8_r = wv8_d.ap().rearrange("(eo p) f -> p eo f", p=P)
    w18_r = w18_d.ap().rearrange("(eo p) f -> p eo f", p=P)
    w2h_r = w2h_d.ap().rearrange("(fo p) e -> p fo e", p=P)
    w28_r = (w28_d.ap().rearrange("(fo p) e -> p fo e", p=P)
             if w28_d is not None else None)
    out_r = out_d.ap().rearrange("(eo p) l -> p eo l", p=P)

    with tile.TileContext(nc) as tc, ExitStack() as stk:
        const = stk.enter_context(tc.tile_pool(name="const", bufs=1))
        px = stk.enter_context(tc.tile_pool(name="px", bufs=1))
        px8 = stk.enter_context(tc.tile_pool(name="px8", bufs=1))
        pvt = stk.enter_context(tc.tile_pool(name="pvt", bufs=1))
        pm = stk.enter_context(tc.tile_pool(name="pm", bufs=1))
        pw1 = stk.enter_context(tc.tile_pool(name="pw1", bufs=1))
        pw2 = stk.enter_context(tc.tile_pool(name="pw2", bufs=1))
        pq = stk.enter_context(tc.tile_pool(name="pq", bufs=2))
        pp = stk.enter_context(tc.tile_pool(name="pp", bufs=2))
        ph = stk.enter_context(tc.tile_pool(name="ph", bufs=2))
        ph8 = stk.enter_context(tc.tile_pool(name="ph8", bufs=2))
        py8 = stk.enter_context(tc.tile_pool(name="py8", bufs=1))
        pysq = stk.enter_context(tc.tile_pool(name="pysq", bufs=1))
        pstat = stk.enter_context(tc.tile_pool(name="pstat", bufs=1))
        py2 = stk.enter_context(tc.tile_pool(name="py2", bufs=1))
        prelu = stk.enter_context(tc.tile_pool(name="prelu", bufs=1))
        pout = stk.enter_context(tc.tile_pool(name="pout", bufs=1))
        paon = stk.enter_context(tc.tile_pool(name="paon", bufs=1))

        ones_f = const.tile([P, P], F32)
        ones3 = const.tile([P, 2, P], F8)
        eps_t = const.tile([P, 1], F32)
        expb_t = const.tile([P, 1], F32)
        b1_t = const.tile([P, FO], F32)
        nc.vector.memset(ones_f[:], 1.0)
        nc.vector.tensor_copy(ones3[:, 0, :], ones_f[:])
        nc.vector.tensor_copy(ones3[:, 1, :], ones_f[:])
        nc.vector.memset(eps_t[:], EPS)
        nc.vector.memset(expb_t[:], EXPB)
        b1_r = b1_d.ap().rearrange("(fo p) -> p fo", p=P)
        b2_t = None
        if b2_d is not None:
            b2_t = const.tile([P, EO], F32)
            nc.sync.dma_start(b2_t[:], b2_d.ap().rearrange("(eo p) -> p eo", p=P))
        ln1w_t = ln1b_t = ln2w_t = ln2b_t = None
        if ln1w_d is not None:
            ln1w_t = const.tile([P, EO], F32)
            ln1b_t = const.tile([P, EO], F32)
            nc.sync.dma_start(ln1w_t[:], ln1w_d.ap().rearrange("(eo p) -> p eo", p=P))
            nc.sync.dma_start(ln1b_t[:], ln1b_d.ap().rearrange("(eo p) -> p eo", p=P))
        if ln2w_d is not None:
            ln2w_t = const.tile([P, EO], F32)
            ln2b_t = const.tile([P, EO], F32)
            nc.sync.dma_start(ln2w_t[:], ln2w_d.ap().rearrange("(eo p) -> p eo", p=P))
            nc.sync.dma_start(ln2b_t[:], ln2b_d.ap().rearrange("(eo p) -> p eo", p=P))

        xt = px.tile([P, EO, L], F32R)          # x^T, becomes y = x + attn in B
        x8 = px8.tile([P, EO, L], F8)           # fp8 x^T for proj/scores
        vt = pvt.tile([P, SB, E], F8)           # v natural [s, e]
        m8 = pm.tile([P, EO, E], F8)            # 128*SCALE*Wq^T Wk
        wv8 = pm.tile([P, EO, E], F8, name="wv8")
        w1t = pw1.tile([P, EO, FF], F8)
        w2t = pw2.tile([P, FO, E], F16)
        w28t = pw2.tile([P, 2 * DR_FO, E], F8, name="w28t") if DR_FO else None
        state = {}

        def stats_dr(s_ps, s2_ps, y8, ysq8):
            """partition sums of y and y^2 from their fp8 shadows (DoubleRow)."""
            for kp in range(EP):
                nc.tensor.matmul(s_ps[:], ones3[:], y8[:, 2 * kp:2 * kp + 2, :],
                                 start=(kp == 0), stop=(kp == EP - 1),
                                 perf_mode=DR)
            for kp in range(EP):
                nc.tensor.matmul(s2_ps[:], ones3[:],
                                 ysq8[:, 2 * kp:2 * kp + 2, :],
                                 start=(kp == 0), stop=(kp == EP - 1),
                                 perf_mode=DR)

        def shadows(i, tag, y3d):
            """fp8 copies of y (DVE) and y^2 (ACT) for the stats matmuls,
            as single 3D-AP ops. gpsimd is NOT used here: its tensor_copy
            costs ~2us per tile and these copies gate the PE's stats
            matmuls."""
            y8 = py8.tile([P, EO, LC], F8, tag="y8", name=f"y8{tag}_{i}")
            ysq8 = pysq.tile([P, EO, LC], F8, tag="ysq", name=f"ysq{tag}_{i}")
            nc.vector.tensor_copy(y8[:], y3d.bitcast(F32))
            nc.scalar.activation(ysq8[:], y3d.bitcast(F32), AF.Square)
            return y8, ysq8

        def ln_stats_rest(i, tag, s_ps, s2_ps, y_sl, w=LC):
            """negmean/meansq on ACT, var+rstd via sqrt + fast reciprocal.
            s_ps/s2_ps are [P, w] APs."""
            negmean = pstat.tile([P, w], F32, tag=f"nm{w}", name=f"nm{tag}_{i}")
            msq = pstat.tile([P, w], F32, tag=f"msq{w}", name=f"msq{tag}_{i}")
            ex2 = pstat.tile([P, w], F32, tag=f"ex2{w}", name=f"ex2{tag}_{i}")
            rstd = pstat.tile([P, w], F32, tag=f"rstd{w}", name=f"rstd{tag}_{i}")
            nc.scalar.activation(negmean[:], s_ps, AF.Copy, scale=-1.0 / E)
            nc.scalar.activation(msq[:], s_ps, AF.Square, scale=1.0 / E)
            nc.vector.tensor_scalar_mul(ex2[:], s2_ps, 1.0 / E)
            nc.vector.tensor_tensor(ex2[:], ex2[:], msq[:], OP.subtract)
            nc.scalar.activation(ex2[:], ex2[:], AF.Sqrt, bias=eps_t[:])
            nc.vector.reciprocal_approx_fast(rstd[:], ex2[:])
            return y_sl, negmean, rstd

        def ln1_apply(i):
            y_sl, negmean, rstd = state.pop(("ln1", i))
            h = ph.tile([P, EO, LC], F32R, tag="h", name=f"h{i}")
            h8 = ph8.tile([P, EO, LC], F8, tag="h8", name=f"h8_{i}")
            for ec in range(EO):
                t = pstat.tile([P, LC], F32, tag="lnapp", name=f"la1_{i}_{ec}")
                nc.vector.tensor_tensor(t[:], y_sl[ec].bitcast(F32),
                                        negmean[:], OP.add)
                if ln1_trivial:
                    nc.vector.tensor_tensor(h[:, ec, :], t[:], rstd[:], OP.mult)
                    nc.vector.tensor_tensor(h8[:, ec, :], t[:], rstd[:], OP.mult)
                else:
                    nc.vector.tensor_tensor(t[:], t[:], rstd[:], OP.mult)
                    nc.scalar.activation(h[:, ec, :], t[:], AF.Identity,
                                         bias=ln1b_t[:, ec:ec + 1],
                                         scale=ln1w_t[:, ec:ec + 1])
                    nc.vector.tensor_copy(h8[:, ec, :], h[:, ec, :].bitcast(F32))
            state[("h", i)] = (h, h8)

        with tc.tile_pool(name="psMM", bufs=2, space="PSUM") as psMM:
            # ------------- DMA program (sync + gpsimd queues only) -------------
            # first two DMAs carry exactly what mq_proj(0)'s first matmul
            # needs, so the PE starts as early as possible
            # first-need pairs split across the two queues so they land
            # concurrently: mq_proj(0)'s kp=0 matmul needs only eo 0-1
            nc.sync.dma_start(m8[:, 0:2, :], m8_r[:, 0:2, :])
            nc.gpsimd.dma_start(x8[:, 0:2, 0:LC], x8_r[:, 0:2, 0:LC])
            nc.sync.dma_start(m8[:, 2:4, :], m8_r[:, 2:4, :])
            nc.gpsimd.dma_start(x8[:, 2:4, 0:LC], x8_r[:, 2:4, 0:LC])
            nc.sync.dma_start(wv8[:], wv8_r)
            for lc in range(1, NLC):
                nc.sync.dma_start(x8[:, :, lc * LC:(lc + 1) * LC],
                                  x8_r[:, :, lc * LC:(lc + 1) * LC])
            nc.gpsimd.dma_start(b1_t[:], b1_r)
            for lc in range(NLC):
                ls = lc * LC
                nc.gpsimd.dma_start(xt[:, :, ls:ls + LC], xt_r[:, :, ls:ls + LC])
            nc.gpsimd.dma_start(w1t[:, :, 0:FF // 2], w18_r[:, :, 0:FF // 2])
            nc.gpsimd.dma_start(w1t[:, :, FF // 2:FF], w18_r[:, :, FF // 2:FF])
            if w28t is not None:
                nc.gpsimd.dma_start(w28t[:], w28_r)
            nc.gpsimd.dma_start(w2t[:, 0:8, :], w2h_r[:, 0:8, :])
            nc.gpsimd.dma_start(w2t[:, 8:16, :], w2h_r[:, 8:16, :])

            # ---------------- phase A/B: attention ----------------
            def mq_proj(lc):
                ls = lc * LC
                mq = pq.tile([P, EO, LC], F8, tag="q", name=f"mq{lc}")
                for fb in range(EO):
                    qp = psMM.tile([P, LC], F32, tag="mm", name=f"qp{lc}_{fb}")
                    for kp in range(EP):
                        nc.tensor.matmul(
                            qp[:], m8[:, 2 * kp:2 * kp + 2, fb * P:(fb + 1) * P],
                            x8[:, 2 * kp:2 * kp + 2, ls:ls + LC],
                            start=(kp == 0), stop=(kp == EP - 1), perf_mode=DR)
                    # DVE evict: the ACT queue is saturated by exp evictions
                    # in phase B, and a late mq evict stalls the next chunk
                    nc.vector.tensor_copy(mq[:, fb, :], qp[:])
                return mq

            qts = {0: mq_proj(0)}

            def vproj(lb):
                """v projection block — interleaved into chunk 0's score
                stream so it runs behind the x8/wv8 DMA instead of stalling
                a separate phase."""
                vp = psMM.tile([P, LC], F32, tag="mm", name=f"vp{lb}")
                for kp in range(EP):
                    nc.tensor.matmul(
                        vp[:], x8[:, 2 * kp:2 * kp + 2, lb * P:(lb + 1) * P],
                        wv8[:, 2 * kp:2 * kp + 2, :],
                        start=(kp == 0), stop=(kp == EP - 1), perf_mode=DR)
                nc.vector.tensor_scalar_mul(vt[:, lb, :], vp[:], 1.0 / WVS)

            with (
                tc.tile_pool(name="paoPS", bufs=4, space="PSUM") as psAO,
                tc.tile_pool(name="psCS", bufs=1, space="PSUM") as psCS,
                tc.tile_pool(name="psSB", bufs=1, space="PSUM") as psSB,
            ):
                # chunk-0 LN1 pieces, injected into chunk 1's attention stream
                def ln1c0_shadows():
                    """both shadows on the DVE here: the ACT is saturated by
                    exp evictions inside the phase-B stream."""
                    y_sl = [xt[:, ec, 0:LC] for ec in range(EO)]
                    y8 = py8.tile([P, EO, LC], F8, tag="y8", name="y81_0")
                    ysq8 = pysq.tile([P, EO, LC], F8, tag="ysq", name="ysq1_0")
                    y3d = xt[:, :, 0:LC]
                    nc.vector.tensor_copy(y8[:], y3d.bitcast(F32))
                    nc.vector.tensor_tensor(ysq8[:], y3d.bitcast(F32),
                                            y3d.bitcast(F32), OP.mult)
                    state["c0"] = (y_sl, (y8, ysq8))

                def ln1c0_sum1():
                    y_sl, (y8, ysq8) = state["c0"]
                    s_ps = psSB.tile([P, LC], F32, tag="sums", name="s1_0")
                    for kp in range(EP):
                        nc.tensor.matmul(s_ps[:], ones3[:],
                                         y8[:, 2 * kp:2 * kp + 2, :],
                                         start=(kp == 0), stop=(kp == EP - 1),
                                         perf_mode=DR)
                    negmean = pstat.tile([P, LC], F32, tag="nm", name="nm1_0")
                    nc.scalar.activation(negmean[:], s_ps[:], AF.Copy,
                                         scale=-1.0 / E)
                    msq = pstat.tile([P, LC], F32, tag="msq", name="msq1_0")
                    nc.scalar.activation(msq[:], s_ps[:], AF.Square, scale=1.0 / E)
                    state["c0b"] = (negmean, msq)

                def ln1c0_sum2():
                    y_sl, (y8, ysq8) = state.pop("c0")
                    negmean, msq = state.pop("c0b")
                    s2_ps = psSB.tile([P, LC], F32, tag="sums", name="s2_0")
                    for kp in range(EP):
                        nc.tensor.matmul(s2_ps[:], ones3[:],
                                         ysq8[:, 2 * kp:2 * kp + 2, :],
                                         start=(kp == 0), stop=(kp == EP - 1),
                                         perf_mode=DR)
                    ex2 = pstat.tile([P, LC], F32, tag="ex2", name="ex21_0")
                    rstd = pstat.tile([P, LC], F32, tag="rstd", name="rstd1_0")
                    nc.vector.tensor_scalar_mul(ex2[:], s2_ps[:], 1.0 / E)
                    nc.vector.tensor_tensor(ex2[:], ex2[:], msq[:], OP.subtract)
                    nc.scalar.activation(ex2[:], ex2[:], AF.Sqrt, bias=eps_t[:])
                    nc.vector.reciprocal_approx_fast(rstd[:], ex2[:])
                    state[("ln1", 0)] = ([xt[:, ec, 0:LC] for ec in range(EO)],
                                         negmean, rstd)

                for lc in range(NLC):
                    ls = lc * LC
                    mq = qts.pop(lc)
                    pexp = pp.tile([P, SB, LC], F8E5, tag="pexp", name=f"pexp{lc}")
                    ao = [psAO.tile([P, LC], F32, tag="ao", name=f"ao{lc}_{e}")
                          for e in range(EO)]
                    cs = psCS.tile([P, LC], F32, tag="cs", name=f"cs{lc}")

                    inject = {}
                    if lc == 1:
                        inject = {0: ln1c0_shadows, 5: ln1c0_sum1}
                    elif lc == 2:
                        inject = {1: ln1c0_sum2, 4: lambda: ln1_apply(0)}

                    def scores2(sb, mq=mq, pexp=pexp, lc=lc):
                        sp = psMM.tile([P, LC], F32, tag="mm",
                                       name=f"sp{lc}_{sb}")
                        for kp in range(EP):
                            nc.tensor.matmul(
                                sp[:], x8[:, 2 * kp:2 * kp + 2,
                                          sb * P:(sb + 1) * P],
                                mq[:, 2 * kp:2 * kp + 2, :],
                                start=(kp == 0), stop=(kp == EP - 1),
                                perf_mode=DR)
                        nc.scalar.activation(pexp[:, sb, :], sp[:], AF.Exp,
                                             bias=expb_t[:], scale=1.0 / MS)

                    def av(j, pexp=pexp, ao=ao, cs=cs):
                        nc.tensor.matmul(cs[:], ones3[:],
                                         pexp[:, 2 * j:2 * j + 2, :],
                                         start=(j == 0), stop=(j == SBP - 1),
                                         perf_mode=DR)
                        for eb in range(EO):
                            nc.tensor.matmul(
                                ao[eb][:],
                                vt[:, 2 * j:2 * j + 2, eb * P:(eb + 1) * P],
                                pexp[:, 2 * j:2 * j + 2, :],
                                start=(j == 0), stop=(j == SBP - 1),
                                perf_mode=DR)

                    if lc == 0:
                        vproj(0)
                        vproj(1)
                    scores2(0)
                    scores2(1)
                    for j in range(SBP):
                        if j + 1 < SBP:
                            if lc == 0:
                                vproj(2 * j + 2)
                                vproj(2 * j + 3)
                            scores2(2 * j + 2)
                            scores2(2 * j + 3)
                        av(j)
                        if j in inject:
                            inject[j]()
                        # next chunk's projection sits mid-chunk so its DVE
                        # evictions finish before this chunk's stream ends
                        if j == 5 and lc + 1 < NLC:
                            qts[lc + 1] = mq_proj(lc + 1)

                    rcs = pstat.tile([P, LC], F32, tag="rcs")
                    nc.vector.reciprocal_approx_fast(rcs[:], cs[:])
                    # y = x + ao * rcs   (in place into xt; one 3D add)
                    aon = paon.tile([P, EO, LC], F32, tag="aon")
                    for ec in range(EO):
                        nc.vector.tensor_tensor(aon[:, ec, :], ao[ec][:],
                                                rcs[:], OP.mult)
                    nc.vector.tensor_tensor(
                        xt[:, :, ls:ls + LC],
                        xt[:, :, ls:ls + LC].bitcast(F32), aon[:], OP.add)

        # ---------------- phase C: LN1, FFN, LN2 per l-chunk ----------------
        with (
            tc.tile_pool(name="psF1", bufs=2, space="PSUM") as psF1,
            tc.tile_pool(name="psF2", bufs=4, space="PSUM") as psF2,
            tc.tile_pool(name="psS", bufs=2, space="PSUM") as psS,
        ):
            def ln_pre_shadows(i):
                """fp8 shadows for chunk i's LN1 — emitted well before the
                stats so the PE never waits on the DVE/ACT queues."""
                ls = i * LC
                state[("ln1sh", i)] = shadows(i, "1", xt[:, :, ls:ls + LC])

            def ln_pre(i):
                """partition sums + rstd for chunk i's LN1."""
                ls = i * LC
                y_sl = [xt[:, ec, ls:ls + LC] for ec in range(EO)]
                y8, ysq8 = state.pop(("ln1sh", i))
                s_ps = psS.tile([P, LC], F32, tag="sums", name=f"s1_{i}")
                s2_ps = psS.tile([P, LC], F32, tag="sums", name=f"s2_{i}")
                stats_dr(s_ps, s2_ps, y8, ysq8)
                state[("ln1", i)] = ln_stats_rest(i, "1", s_ps[:], s2_ps[:], y_sl)

            # FFN2 consumption units: DR_FO fp8 pairs, then single f16 fo's
            NU = FO - DR_FO
            def unit_last_fo(k):
                return 2 * k + 1 if k < DR_FO else k + DR_FO

            def ffn_start(i):
                relu8 = (prelu.tile([P, 2 * DR_FO, LC], F8, tag="relu8",
                                    name=f"relu8_{i}") if DR_FO else None)
                relu16 = prelu.tile([P, FO - 2 * DR_FO, LC], F16, tag="relu16",
                                    name=f"relu16_{i}")
                ao2 = [psF2.tile([P, LC], F32, tag="ao2", name=f"ao2_{i}_{e}")
                       for e in range(EO)]
                state[("ffn", i)] = (relu8, relu16, ao2)

            def ffn1(i, fo):
                relu8, relu16, _ = state[("ffn", i)]
                _, h8 = state[("h", i)]
                fp = psF1.tile([P, LC], F32, tag="f1", name=f"fp{i}_{fo}")
                for kp in range(EP):
                    nc.tensor.matmul(
                        fp[:], w1t[:, 2 * kp:2 * kp + 2, fo * P:(fo + 1) * P],
                        h8[:, 2 * kp:2 * kp + 2, :],
                        start=(kp == 0), stop=(kp == EP - 1), perf_mode=DR)
                dst = (relu8[:, fo, :] if fo < 2 * DR_FO
                       else relu16[:, fo - 2 * DR_FO, :])
                nc.scalar.activation(dst, fp[:], AF.Relu,
                                     bias=b1_t[:, fo:fo + 1])

            def ffn2(i, k):
                relu8, relu16, ao2 = state[("ffn", i)]
                if k < DR_FO:
                    for eb in range(EO):
                        nc.tensor.matmul(
                            ao2[eb][:],
                            w28t[:, 2 * k:2 * k + 2, eb * P:(eb + 1) * P],
                            relu8[:, 2 * k:2 * k + 2, :],
                            start=(k == 0), stop=(k == NU - 1), perf_mode=DR)
                else:
                    fo = k + DR_FO
                    for eb in range(EO):
                        nc.tensor.matmul(
                            ao2[eb][:], w2t[:, fo, eb * P:(eb + 1) * P],
                            relu16[:, fo - 2 * DR_FO, :],
                            start=(k == 0), stop=(k == NU - 1))

            def resid2(i, c0=0, c1=LC, sfx=""):
                """z = h + ffn*2^-5 (+b2) on columns [c0,c1), fp8 shadows +
                partition sums, per-ec interleaved so the post-FFN critical
                path is short. The last chunk runs this in two half-width
                pieces so the DVE/ACT/PE stages pipeline into the tail."""
                ao2 = state[("ffn", i)][-1]
                h, _ = state[("h", i)]
                w = c1 - c0
                if b2_t is not None:
                    for ec in range(EO):
                        nc.vector.tensor_tensor(
                            h[:, ec, c0:c1], h[:, ec, c0:c1].bitcast(F32),
                            b2_t[:, ec:ec + 1].to_broadcast((P, w)), OP.add)
                if c0 == 0:
                    state[("y2t", i)] = (
                        py2.tile([P, EO, LC], F32R, tag="y2", name=f"y2_{i}"),
                        py8.tile([P, EO, LC], F8, tag="y8", name=f"y82_{i}"),
                        pysq.tile([P, EO, LC], F8, tag="ysq", name=f"ysq2_{i}"))
                y2, y8, ysq8 = state[("y2t", i)]
                for ec in range(EO):
                    nc.vector.affine_then_add(y2[:, ec, c0:c1], ao2[ec][:, c0:c1],
                                              h[:, ec, c0:c1].bitcast(F32),
                                              scale=RESID_SCALE, bias=0.0)
                nc.vector.tensor_copy(y8[:, :, c0:c1],
                                      y2[:, :, c0:c1].bitcast(F32))
                nc.scalar.activation(ysq8[:, :, c0:c1],
                                     y2[:, :, c0:c1].bitcast(F32), AF.Square)
                state[("y2sh", i, c0)] = (y2, y8, ysq8, c0, c1, sfx)
                if c1 == LC:
                    state.pop(("h", i))
                    state.pop(("ffn", i))
                    state.pop(("y2t", i))

            def resid2_stats(i, c0=0):
                """the PE half of resid2 — deferred into the next chunk's
                FFN stream so the PE never waits on the DVE shadow chain."""
                y2, y8, ysq8, c0, c1, sfx = state.pop(("y2sh", i, c0))
                w = c1 - c0
                s_ps = psS.tile([P, LC], F32, tag="sums", name=f"s3_{i}{sfx}")
                s2_ps = psS.tile([P, LC], F32, tag="sums", name=f"s4_{i}{sfx}")
                for kp in range(EP):
                    nc.tensor.matmul(s_ps[:, 0:w], ones3[:],
                                     y8[:, 2 * kp:2 * kp + 2, c0:c1],
                                     start=(kp == 0), stop=(kp == EP - 1),
                                     perf_mode=DR)
                for kp in range(EP):
                    nc.tensor.matmul(s2_ps[:, 0:w], ones3[:],
                                     ysq8[:, 2 * kp:2 * kp + 2, c0:c1],
                                     start=(kp == 0), stop=(kp == EP - 1),
                                     perf_mode=DR)
                state[("y2", i, c0)] = (y2, s_ps, s2_ps)

            def ln2_full(i, c0=0, c1=LC, sfx=""):
                y2, s_ps, s2_ps = state.pop(("y2", i, c0))
                w = c1 - c0
                y2_sl = [y2[:, ec, c0:c1] for ec in range(EO)]
                _, negmean, rstd = ln_stats_rest(f"{i}{sfx}", "2",
                                                 s_ps[:, 0:w], s2_ps[:, 0:w],
                                                 y2_sl, w=w)
                ls = i * LC
                if c0 == 0:
                    state[("out", i)] = pout.tile([P, EO, LC], F32, tag="out",
                                                  name=f"out{i}")
                outt = state[("out", i)]
                if c1 == LC:
                    state.pop(("out", i))
                # subtracts only need negmean — they hide under the ACT sqrt
                # and DVE reciprocal that produce rstd
                ts = []
                for ec in range(EO):
                    t = pstat.tile([P, w], F32, tag=f"lnapp{ec}_{w}",
                                   name=f"la2_{i}_{ec}{sfx}")
                    nc.vector.tensor_tensor(t[:], y2_sl[ec].bitcast(F32),
                                            negmean[:], OP.add)
                    ts.append(t)
                for ec in range(EO):
                    t = ts[ec]
                    if ln2_trivial:
                        nc.vector.tensor_tensor(outt[:, ec, c0:c1], t[:],
                                                rstd[:], OP.mult)
                    else:
                        nc.vector.tensor_tensor(t[:], t[:], rstd[:], OP.mult)
                        nc.scalar.activation(outt[:, ec, c0:c1], t[:],
                                             AF.Identity,
                                             bias=ln2b_t[:, ec:ec + 1],
                                             scale=ln2w_t[:, ec:ec + 1])
                    if w < LC:
                        # tail halves stream out per-ec for latency
                        nc.sync.dma_start(out_r[:, ec, ls + c0:ls + c1],
                                          outt[:, ec, c0:c1])
                if w == LC:
                    nc.sync.dma_start(out_r[:, :, ls:ls + LC], outt[:])

            # ---- pipelined emission (h(0) already computed in phase B) ----
            for i in range(NLC):
                ffn_start(i)
                u = 0
                for fo in range(FO):
                    ffn1(i, fo)
                    if fo == 2 and i > 0:
                        resid2_stats(i - 1)  # PE half of resid2, post-shadows
                    if fo == 3 and i > 0:
                        ln2_full(i - 1)      # PE: stats DR MMs amid FFN stream
                    if fo == 4 and i + 1 < NLC:
                        ln_pre_shadows(i + 1)
                    if fo == 8 and i + 1 < NLC:
                        ln_pre(i + 1)        # next chunk's LN1 stats
                    if fo == 11 and i + 1 < NLC:
                        ln1_apply(i + 1)     # h(i+1) ready before FFN(i) ends
                    while u < NU and unit_last_fo(u) <= fo - 2:
                        ffn2(i, u)
                        u += 1
                while u < NU:
                    ffn2(i, u)
                    u += 1
                if i < NLC - 1:
                    resid2(i)
                else:
                    resid2(i, 0, LC // 2, "a")
                    resid2(i, LC // 2, LC, "b")
            resid2_stats(NLC - 1, 0)
            ln2_full(NLC - 1, 0, LC // 2, "a")
            resid2_stats(NLC - 1, LC // 2)
            ln2_full(NLC - 1, LC // 2, LC, "b")

    nc.compile()
    return nc


def kernel(x, in_proj_w, ln1_w, ln1_b, ln2_w, ln2_b, w1, b1, w2, b2):
    global LAST_RESULT
    x = np.asarray(x, dtype=np.float32)
    in_proj_w = np.asarray(in_proj_w, dtype=np.float32)
    w1 = np.asarray(w1, dtype=np.float32)
    w2 = np.asarray(w2, dtype=np.float32)
    b1 = np.asarray(b1, dtype=np.float32)
    b2 = np.asarray(b2, dtype=np.float32)
    ln1_w = np.asarray(ln1_w, dtype=np.float32)
    ln1_b = np.asarray(ln1_b, dtype=np.float32)
    ln2_w = np.asarray(ln2_w, dtype=np.float32)
    ln2_b = np.asarray(ln2_b, dtype=np.float32)

    ln1_trivial = bool(np.all(ln1_w == 1.0) and np.all(ln1_b == 0.0))
    ln2_trivial = bool(np.all(ln2_w == 1.0) and np.all(ln2_b == 0.0))
    b2_zero = bool(np.all(b2 == 0.0))

    key = (ln1_trivial, ln2_trivial, b2_zero)
    if key not in _CACHE:
        _CACHE[key] = _build(*key)
    nc = _CACHE[key]

    wq = in_proj_w[:E].astype(np.float64)
    wk = in_proj_w[E:2 * E].astype(np.float64)
    m8 = _to_f8(MS * SCALE * (wq.T @ wk))                 # [E, E]
    wv8 = _to_f8(WVS * in_proj_w[2 * E:].T)               # [E, E]
    w18 = _to_f8(W1S * w1.T)                              # [E, FF]
    w2t_host = W2S * w2.T                                 # [FF, E]
    w2h = w2t_host.astype(np.float16)
    w28 = _to_f8(w2t_host[:2 * DR_FO * P]) if DR_FO else None
    b1s = (W1S * b1).astype(np.float32)

    in_maps = []
    for bb in range(B):
        xtb = x[bb].T
        m = {
            "xt": _round_fp32r(xtb),                      # [E, L]
            "x8": _to_f8(xtb),
            "m8": m8, "wv8": wv8,
            "w18": w18, "w2h": w2h, "b1v": b1s,
        }
        if w28 is not None:
            m["w28"] = w28
        if not b2_zero:
            m["b2v"] = b2
        if not ln1_trivial:
            m["ln1w"] = ln1_w
            m["ln1b"] = ln1_b
        if not ln2_trivial:
            m["ln2w"] = ln2_w
            m["ln2b"] = ln2_b
        in_maps.append(m)

    res = run_bass_kernel_spmd(nc, in_maps, list(range(B)), trace=_TRACE)
    LAST_RESULT = res
    out = np.stack([np.ascontiguousarray(res.results[bb]["outt"].T)
                    for bb in range(B)])
    return out.astype(np.float32)
